# revision 9
# baseline (speedup 1.0000x reference)
"""Causal multi-head attention block (GPT-style) on 8 TRN2 NeuronCores.

Sharding: core (b, g) = batch b in {0,1} x head-group g in {0..3} (4 heads of
dh=64 each). Megatron-style: each core computes q/k/v projections for its 256
channels, attention for its 4 heads, and a partial c_proj using its 256 rows of
W_proj. Host sums the 4 partial projections per batch (+ bias terms).

All matmul inputs are bf16 (PSUM accumulation stays fp32), which lets every
matmul run at 1 cycle/row regardless of output free size. On-core dataflow:
  qT,kT = (W_qk stationary) @ xT        -> [512, 2048] bf16 (q pre-scaled 1/8)
  v     = (xT stationary) @ W_v         -> [2048, 4*65] bf16 (ones col appended)
  sT    = kT_tile.T @ qT_slice          -> scores transposed [j, i] (PSUM f32)
  u     = exp(sT) in bf16 (no max-subtraction; scores are O(3); causal tiles
          only, diagonal masked by a [128,128] triangle multiply)
  av    = (u stationary) @ (v|1)        -> [128 q, 4*65] PSUM: per head 64
          unnormalized out channels + softmax sum
  a     = av[:, h*65:h*65+64] * recip(av[:, h*65+64])  (per-partition scalar)
  aT    = PE-transpose(a)               -> [ch, q] bf16 for c_proj lhsT
  out   = (aT stationary) @ W_proj_rows -> partial [2048, 1024] bf16

Emission uses a cost-cursor greedy: score rounds for all four 512-token
q-slices form one global pipeline (the ACT exp stream is the pacing
constraint); qkv projections of later slices and c_proj of finished q-tiles
are drained as PE filler exactly up to each round's score-psum ring barrier,
so the PE never waits on the ACT engine.
"""

import sys

try:
    import concourse  # noqa: F401
except ImportError:
    sys.path.insert(0, "/opt/trn_rl_repo")

from contextlib import ExitStack

import numpy as np

import concourse.tile as tile
from concourse import bacc, masks, mybir
from concourse.bass_utils import run_bass_kernel_spmd

F32 = mybir.dt.float32
BF16 = mybir.dt.bfloat16
EXP = mybir.ActivationFunctionType.Exp
COPY = mybir.ActivationFunctionType.Copy
MUL = mybir.AluOpType.mult
ADD = mybir.AluOpType.add

B, T, D = 2, 2048, 1024
HG, DH = 4, 64          # heads per core, head dim
CQK = 512               # q+k channels per core
CV = 256                # v channels per core
KT = D // 128           # contraction tiles of the projections
TS = 512                # t-slice width
NTS = T // TS
NT128 = T // 128
VW = DH + 1             # 65: per-head v row (64 v + 1 ones)

PE_NS = 1e9 / 2.4e9     # full-speed PE ns/row

UNIT_RANGES = []        # (label, first_inst_id) — emission-order unit log


def _mark(nc, label):
    # consumes one instruction id as a position marker (names may skip ids)
    UNIT_RANGES.append((label, nc._state.next_id()))


def build():
    nc = bacc.Bacc(None)

    xT_in = nc.dram_tensor("xT", [NTS, KT, 128, TS], BF16, kind="ExternalInput")
    wqk_in = nc.dram_tensor("wqk", [KT, 128, CQK], BF16, kind="ExternalInput")
    wv_in = nc.dram_tensor("wv", [KT, 128, CV], BF16, kind="ExternalInput")
    wp_in = nc.dram_tensor("wp", [2, 128, D], BF16, kind="ExternalInput")
    bias_in = nc.dram_tensor("bqk", [128, 4], F32, kind="ExternalInput")
    mask_in = nc.dram_tensor("mask", [128, 512], BF16, kind="ExternalInput")
    out_dram = nc.dram_tensor("out", [NT128, 128, 2 * TS], BF16,
                              kind="ExternalOutput")

    with ExitStack() as ctx:
        tc = ctx.enter_context(tile.TileContext(nc))

        const = ctx.enter_context(tc.tile_pool(name="const", bufs=1))
        big = ctx.enter_context(tc.tile_pool(name="big", bufs=1))
        up2 = ctx.enter_context(tc.tile_pool(name="up2", bufs=2))
        up1 = ctx.enter_context(tc.tile_pool(name="up1", bufs=1))
        stp = ctx.enter_context(tc.tile_pool(name="stp", bufs=2))
        rcp = ctx.enter_context(tc.tile_pool(name="rcp", bufs=2))
        outp = ctx.enter_context(tc.tile_pool(name="outp", bufs=3))
        xrp = ctx.enter_context(tc.tile_pool(name="xrp", bufs=1))
        xbp = ctx.enter_context(tc.tile_pool(name="xbp", bufs=2))
        wqkp = ctx.enter_context(tc.tile_pool(name="wqkp", bufs=1))
        wvp = ctx.enter_context(tc.tile_pool(name="wvp", bufs=1))

        ps_s = ctx.enter_context(tc.tile_pool(name="ps_s", bufs=2, space="PSUM"))
        ps_av = ctx.enter_context(tc.tile_pool(name="ps_av", bufs=2, space="PSUM"))
        ps_mm = ctx.enter_context(tc.tile_pool(name="ps_mm", bufs=2, space="PSUM"))

        # constants (DMAs for bias/tri are emitted after the first x/w tiles
        # so they don't delay the critical first matmul)
        bias_sb = const.tile([128, 4], F32, tag="bias")
        tri = const.tile([128, 512], BF16, tag="tri")  # [zeros|tri, zeros|tri]
        ones128 = const.tile([128, 16], BF16, tag="ones128")
        nc.vector.memset(ones128[:], 1.0)
        ident = const.tile([128, 128], BF16, tag="ident")
        masks.make_identity(nc, ident[:])

        # persistent intermediates
        qkT = {(ct, ts): big.tile([128, TS], BF16, tag=f"qkT{ct}_{ts}",
                                  name=f"qkT{ct}_{ts}")
               for ct in range(4) for ts in range(NTS)}
        Vt = [big.tile([128, 4 * HG * VW], BF16, tag=f"Vt{ts}", name=f"Vt{ts}")
              for ts in range(NTS)]
        aTT = [big.tile([128, T], BF16, tag=f"aTT{c}", name=f"aTT{c}")
               for c in range(2)]
        wp = [big.tile([128, D], BF16, tag=f"wp{c}", name=f"wp{c}")
              for c in range(2)]

        # startup DMAs: few big blobs (HWDGE descriptor-gen is serialized at
        # 625ns/DMA): W_qk whole, slice-0 x in two halves, then wv blob,
        # bias/tri, W_proj.
        xrb = {}
        wqkb = wqkp.tile([128, KT * CQK], BF16, tag="wqk", name="wqk")
        nc.sync.dma_start(
            out=wqkb[:].rearrange("p (k c) -> k p c", k=KT), in_=wqk_in[:])
        wqk = [wqkb[:, k * CQK:(k + 1) * CQK] for k in range(KT)]
        xb0 = xrp.tile([128, KT * TS], BF16, tag="xr0", name="xr_0")
        xrb[0] = xb0
        h = KT // 2
        x3 = xb0[:].rearrange("p (k c) -> k p c", k=KT)
        nc.sync.dma_start(out=x3[0:h], in_=xT_in[0][0:h])
        nc.sync.dma_start(out=x3[h:KT], in_=xT_in[0][h:KT])
        wvb = wvp.tile([128, KT * CV], BF16, tag="wv", name="wv")
        nc.sync.dma_start(
            out=wvb[:].rearrange("p (k c) -> k p c", k=KT), in_=wv_in[:])
        wv = [wvb[:, k2 * CV:(k2 + 1) * CV] for k2 in range(KT)]
        nc.sync.dma_start(out=bias_sb[:], in_=bias_in[:])
        nc.sync.dma_start(out=tri[:], in_=mask_in[:])
        for c in range(2):
            nc.sync.dma_start(out=wp[c][:], in_=wp_in[c])

        def xr_ap(ts, k):
            return xrb[ts][:, k * TS:(k + 1) * TS]

        def load_xr(ts):
            def unit():
                _mark(nc, f"load_xr{ts}")
                xb = xbp.tile([128, KT * TS], BF16, tag="xrb",
                              name=f"xr_{ts}")
                nc.sync.dma_start(
                    out=xb[:].rearrange("p (k c) -> k p c", k=KT),
                    in_=xT_in[ts])
                xrb[ts] = xb
            return unit

        def qk_group(ts, ct):
            def unit():
                _mark(nc, f"qk_{ts}_{ct}")
                ps = ps_mm.tile([128, TS], F32, tag="mm", name=f"qk_{ts}_{ct}")
                for k in range(KT):
                    nc.tensor.matmul(ps[:], wqk[k][:, ct * 128:(ct + 1) * 128],
                                     xr_ap(ts, k), start=(k == 0),
                                     stop=(k == KT - 1))
                scale = 0.125 if ct < 2 else 1.0
                nc.vector.tensor_scalar(
                    qkT[ct, ts][:], ps[:],
                    scale, bias_sb[:, ct:ct + 1], op0=MUL, op1=ADD)
            return unit

        def v_group(ts, sub):
            def unit():
                _mark(nc, f"v_{ts}_{sub}")
                ps = ps_mm.tile([128, CV], F32, tag="mm", name=f"v_{ts}_{sub}")
                for k in range(KT):
                    nc.tensor.matmul(ps[:],
                                     xr_ap(ts, k)[:, sub * 128:(sub + 1) * 128],
                                     wv[k], start=(k == 0), stop=(k == KT - 1))
                v3 = Vt[ts][:].rearrange("p (s h e) -> p s h e", h=HG, e=VW)
                nc.gpsimd.tensor_copy(
                    v3[:, sub, :, 0:DH],
                    ps[:].rearrange("p (h e) -> p h e", e=DH))
                if sub == 0:
                    nc.vector.tensor_copy(
                        v3[:, :, :, DH],
                        ones128[:].rearrange("p (s h) -> p s h", h=HG))
            return unit

        utiles = {}

        def sc_unit(gi, hp, jt):
            def unit():
                _mark(nc, f"sc_{gi}_{hp}_{jt}")
                d = jt * 128 - gi * TS
                c0 = max(d, 0)
                ss = ps_s.tile([128, 2 * TS], F32, tag="ss",
                               name=f"ss_{gi}_{hp}_{jt}")
                jts = jt // 4
                jo = (jt % 4) * 128
                for half in range(2):
                    p0 = half * 64
                    nc.tensor.matmul(
                        ss[:, half * TS + c0:(half + 1) * TS],
                        qkT[2 + hp, jts][p0:p0 + 64, jo:jo + 128],
                        qkT[hp, gi][p0:p0 + 64, c0:TS],
                        start=True, stop=True)
                pool = up2 if jt < 8 else up1
                u = pool.tile([128, 2 * TS], BF16, tag=f"u{hp}_{jt}",
                              name=f"u_{gi}_{hp}_{jt}")
                utiles[gi, hp, jt] = u
                u3 = u[:].rearrange("p (h i) -> p h i", h=2)
                s3 = ss[:].rearrange("p (h i) -> p h i", h=2)
                if c0:
                    nc.scalar.activation(u3[:, :, c0:TS], s3[:, :, c0:TS], EXP)
                else:
                    nc.scalar.activation(u[:], ss[:], EXP)
                if d >= 0:
                    # triangle-mask columns [d, d+128) of the diagonal tile
                    t3 = tri[:].rearrange("p (h m) -> p h m", h=2)
                    nc.vector.tensor_tensor(
                        u3[:, :, d:d + 128], u3[:, :, d:d + 128],
                        t3[:, :, 128:256], op=MUL)
            return unit

        avps = {}

        def av_unit(gi, qt):
            qtg = 4 * gi + qt

            def unit():
                _mark(nc, f"av_{gi}_{qt}")
                avp = ps_av.tile([128, HG * VW], F32, tag="av",
                                 name=f"av_{gi}_{qt}")
                avps[gi, qt] = avp
                for h in range(HG):
                    hp, half = h // 2, h % 2
                    for jt in range(qtg + 1):
                        u = utiles[gi, hp, jt]
                        v3 = Vt[jt // 4][:].rearrange(
                            "p (s h e) -> p s h e", h=HG, e=VW)
                        nc.tensor.matmul(
                            avp[:, h * VW:(h + 1) * VW],
                            u[:, half * TS + qt * 128:half * TS + (qt + 1) * 128],
                            v3[:, jt % 4, h, :],
                            start=(jt == 0), stop=(jt == qtg))
            return unit

        stages = {}

        def norm_unit(gi, qt):
            def unit():
                _mark(nc, f"norm_{gi}_{qt}")
                avp = avps[gi, qt]
                av3 = avp[:].rearrange("p (h e) -> p h e", e=VW)
                rc = rcp.tile([128, 4], F32, tag="rc", name=f"rc_{gi}_{qt}")
                nc.vector.reciprocal(rc[:], av3[:, :, DH])
                st = stp.tile([128, 2 * 128], BF16, tag="st",
                              name=f"st_{gi}_{qt}")
                stages[gi, qt] = st
                for h in range(HG):
                    nc.vector.tensor_scalar(
                        st[:, h * DH:(h + 1) * DH], av3[:, h, 0:DH],
                        rc[:, h:h + 1], None, op0=MUL)
            return unit

        def tp_unit(gi, qt):
            qtg = 4 * gi + qt

            def unit():
                _mark(nc, f"tp_{gi}_{qt}")
                st = stages[gi, qt]
                if gi == NTS - 1 and qt >= 2:
                    # tail: PE transpose (short latency; mm ring is free now)
                    tp = ps_mm.tile([128, 2 * 128], BF16, tag="mm",
                                    name=f"tp_{gi}_{qt}")
                    for c in range(2):
                        nc.tensor.transpose(
                            tp[:, c * 128:(c + 1) * 128],
                            st[:, c * 128:(c + 1) * 128], ident[:])
                    for c in range(2):
                        nc.vector.tensor_copy(
                            aTT[c][:, qtg * 128:(qtg + 1) * 128],
                            tp[:, c * 128:(c + 1) * 128])
                else:
                    # steady state: DMA xbar transpose straight into aTT
                    for c in range(2):
                        nc.sync.dma_start_transpose(
                            out=aTT[c][:, qtg * 128:(qtg + 1) * 128],
                            in_=st[:, c * 128:(c + 1) * 128])
            return unit

        def proj_unit(tt):
            def unit():
                _mark(nc, f"proj_{tt}")
                o = outp.tile([128, 2 * TS], BF16, tag="o", name=f"o_{tt}")
                for nt in range(2):
                    ps = ps_mm.tile([128, TS], F32, tag="mm",
                                    name=f"pj_{tt}_{nt}")
                    for c in range(2):
                        nc.tensor.matmul(
                            ps[:], aTT[c][:, tt * 128:(tt + 1) * 128],
                            wp[c][:, nt * TS:(nt + 1) * TS],
                            start=(c == 0), stop=(c == 1))
                    nc.gpsimd.tensor_copy(o[:, nt * TS:(nt + 1) * TS], ps[:])
                nc.sync.dma_start(out=out_dram[tt], in_=o[:])
            return unit

        # ---- cost-cursor greedy emission -------------------------------
        QK_PE = 8 * TS * PE_NS
        V_PE = 8 * CV * PE_NS
        PROJ_PE = 4 * TS * PE_NS
        TP_PE = 2 * 128 * PE_NS
        ACT_OV = 320.0          # per-exp-instruction overhead (access + sem)
        ACT_LAG = 150.0         # score-psum sem -> exp start latency

        rounds = [(gi, jt) for gi in range(NTS) for jt in range(4 * gi + 4)]
        ridx_of = {r: i for i, r in enumerate(rounds)}

        # filler queue entries: (deadline_ridx, pe_cost, unit)
        filler = []

        def drain(barrier_ns, pe_t, now_ridx):
            while filler:
                dl, cost, u = filler[0]
                if dl > now_ridx and pe_t >= barrier_ns:
                    break
                filler.pop(0)
                u()
                pe_t += cost
            return pe_t

        # phase A, slice 0: fully sequential (nothing to overlap with)
        pe_t = 0.0
        for ct in (2, 3, 0, 1):
            qk_group(0, ct)()
            pe_t += QK_PE
        for sub in range(4):
            v_group(0, sub)()
            pe_t += V_PE

        act_t = 0.0
        act_hist = []
        trail = []
        for ridx, (gi, jt) in enumerate(rounds):
            if jt == 0:
                # enqueue next slice's phase A as filler: q-projections are
                # due before that slice's first round, k/v before its
                # diagonal rounds; x loads start now (DMA latency hiding)
                if gi + 1 < NTS:
                    load_xr(gi + 1)()
                    nslc = gi + 1
                    dl_q = ridx_of[nslc, 0]
                    dl_kv = ridx_of[nslc, 4 * nslc]
                    filler.append((dl_q, QK_PE, qk_group(nslc, 0)))
                    filler.append((dl_q, QK_PE, qk_group(nslc, 1)))
                    filler.append((dl_kv, QK_PE, qk_group(nslc, 2)))
                    filler.append((dl_kv, QK_PE, qk_group(nslc, 3)))
                    for sub in range(4):
                        filler.append((dl_kv, V_PE, v_group(nslc, sub)))
            # score-psum ring barrier: this round's matmuls wait on the exp
            # two rounds back
            if ridx >= 2:
                pe_t = max(pe_t, act_hist[ridx - 2])
            d = jt * 128 - gi * TS
            c0 = max(d, 0)
            sc_pe = 2 * 2 * (TS - c0) * PE_NS
            sc_unit(gi, 0, jt)()
            sc_unit(gi, 1, jt)()
            pe_t += sc_pe
            act_t = max(act_t, pe_t + ACT_LAG) \
                + 2 * (2 * (TS - c0) * 0.8333 + ACT_OV)
            act_hist.append(act_t)
            m = jt - 4 * gi
            if m >= 0:
                av_unit(gi, m)()
                pe_t += 4 * (4 * gi + m + 1) * VW * PE_NS
                # norm trails av by one round, tp trails norm by one more:
                # each crosses an engine (PE->DVE->PE), so give the
                # consumer a round of slack instead of stalling on it
                if m >= 1:
                    norm_unit(gi, m - 1)()
                if m >= 2:
                    tp_unit(gi, m - 2)()
                    pe_t += TP_PE
                    filler.append((10 ** 9, PROJ_PE, proj_unit(4 * gi + m - 2)))
                if m == 3:
                    norm_unit(gi, 3)()
            barrier = act_hist[ridx - 1] if ridx >= 1 else 0.0
            pe_t = drain(barrier, pe_t, ridx + 1)
            # cross-slice trailing tp units ride at the next two round-tops
            if m == 3:
                trail.append((gi, 2))
                trail.append((gi, 3))
            elif trail:
                tgi, tqt = trail.pop(0)
                tp_unit(tgi, tqt)()
                pe_t += TP_PE
                filler.append((10 ** 9, PROJ_PE, proj_unit(4 * tgi + tqt)))
        for tgi, tqt in trail:
            tp_unit(tgi, tqt)()
            filler.append((10 ** 9, PROJ_PE, proj_unit(4 * tgi + tqt)))
        # flush remaining filler (late c_proj tiles)
        while filler:
            _, _, u = filler.pop(0)
            u()

    nc.finalize()
    return nc


_NC = None


def _get_nc():
    global _NC
    if _NC is None:
        _NC = build()
    return _NC


def _bf16(a):
    import ml_dtypes
    return np.asarray(a, dtype=np.float32).astype(ml_dtypes.bfloat16)


def _make_in_maps(x, W_attn, b_attn, W_proj):
    jj = np.arange(128, dtype=np.int64)[:, None]
    ii = np.arange(128, dtype=np.int64)[None, :]
    tri = (jj <= ii).astype(np.float32)
    zt = np.concatenate([np.zeros((128, 128), np.float32), tri], axis=1)
    mask = _bf16(np.concatenate([zt, zt], axis=1))

    shards = []
    for g in range(4):
        q_cols = W_attn[:, g * CV:(g + 1) * CV]
        k_cols = W_attn[:, D + g * CV:D + (g + 1) * CV]
        wqk = _bf16(np.ascontiguousarray(
            np.concatenate([q_cols, k_cols], axis=1)).reshape(KT, 128, CQK))
        wv = _bf16(np.ascontiguousarray(
            W_attn[:, 2 * D + g * CV:2 * D + (g + 1) * CV]).reshape(KT, 128, CV))
        wp = _bf16(np.ascontiguousarray(
            W_proj[g * CV:(g + 1) * CV, :]).reshape(2, 128, D))
        bq = b_attn[g * CV:(g + 1) * CV] / 8.0
        bk = b_attn[D + g * CV:D + (g + 1) * CV]
        bqk = np.ascontiguousarray(
            np.concatenate([bq, bk]).reshape(4, 128).T).astype(np.float32)
        shards.append((wqk, wv, wp, bqk))

    in_maps = []
    for b in range(B):
        xT = np.ascontiguousarray(x[b].T).reshape(KT, 128, NTS, TS)
        xT = _bf16(np.ascontiguousarray(xT.transpose(2, 0, 1, 3)))
        for g in range(4):
            wqk, wv, wp, bqk = shards[g]
            in_maps.append({
                "xT": xT, "wqk": wqk, "wv": wv, "wp": wp,
                "bqk": bqk, "mask": mask,
            })
    return in_maps


def run(inputs, trace=False):
    x = np.asarray(inputs["x"], dtype=np.float32)
    W_attn = np.asarray(inputs["W_attn"], dtype=np.float32)
    b_attn = np.asarray(inputs["b_attn"], dtype=np.float32)
    W_proj = np.asarray(inputs["W_proj"], dtype=np.float32)
    b_proj = np.asarray(inputs["b_proj"], dtype=np.float32)

    nc = _get_nc()
    in_maps = _make_in_maps(x, W_attn, b_attn, W_proj)
    res = run_bass_kernel_spmd(nc, in_maps, list(range(8)), trace=trace)

    out = np.zeros((B, T, D), dtype=np.float32)
    for b in range(B):
        for g in range(4):
            o = np.asarray(res.results[b * 4 + g]["out"], dtype=np.float32)
            out[b] += o.reshape(T, D)
    # v-bias contributes a constant shift through the value path; b_proj too.
    const = b_attn[2 * D:3 * D] @ W_proj + b_proj
    out += const[None, None, :].astype(np.float32)
    return out, res


def kernel(**inputs):
    out, _ = run(inputs, trace=False)
    return out
